# revision 9
# baseline (speedup 1.0000x reference)
"""GNN message-passing layer (nn_DEMOWeightLayer) on 8 Trainium2 NeuronCores.

Reference computation (per node i):
    out = elu(x @ Wg.T + [deg>0] * (mean_{src->i} x[src] @ Wl.T + x @ Ws.T) + b)

Sharding: nodes (and their incoming edges, since dst is sorted) are split
contiguously across 8 cores; x is replicated to every core's HBM so no
cross-core exchange is needed.  Per core, everything is computed in a
feature-on-partitions (transposed) layout:

  - x[src] rows (f16, 256B) are fetched with dma_gather (SWDGE, 4 queues
    for parallel Q7 descriptor generation).  src indices are int16, so
    edges are split into two streams by src < 32768 (table A = x[:32768],
    B = x[32768:]).
  - segment-sum becomes a one-hot matmul per 128-edge chunk:
        psum[feat, 128 nodes] += chunk[128e, feat].T @ onehot[128e, 128n]
    with onehot[e, n] = (dst_local[e] == n) * (1/deg[dst[e]]).  One-hots
    for all 16 chunks of a gather call are built in two batched DVE ops
    using stride-0 broadcast access patterns.
  - edges of each (128-node block, half) are padded to whole chunks with
    (idx=0, dl=-1, w=0); chunk counts per (block, half) are the max over
    the 8 cores so one program serves all cores (SPMD).
  - dense branch per 512-node group: Wg@xT + Ws@xTm + Wl@nm + b*1 in PSUM,
    then ELU = (max(y,0)-1) + exp(min(y,0)) via DVE+ACT.
"""

import sys
import numpy as np

sys.path.insert(0, "/opt/trn_rl_repo")

N = 50000
D = 128
NCORES = 8
NPC = N // NCORES            # 6250 nodes per core
BLK = 128
NBLK = (NPC + BLK - 1) // BLK  # 49 blocks (6272 padded nodes)
NPAD = NBLK * BLK
SPLIT = 32768                # int16 index limit
GRP = 512                    # dense-phase node group (one PSUM bank)
IDX_PER_CALL = 2048          # per dma_gather call (16 chunks)
CPC = IDX_PER_CALL // 128    # chunks per call
NQUEUES = 4

_cache = {}


def _build_host_data(x, Wg, Wl, Ws, b, src, dst, deg):
    """Shard + pad the edge streams; returns (structure, per-core in_maps)."""
    x16 = np.asarray(x, np.float32).astype(np.float16)
    deg = np.asarray(deg, np.int64)
    src = np.asarray(src, np.int64)
    dst = np.asarray(dst, np.int64)

    cum = np.concatenate([[0], np.cumsum(deg)])
    invdeg = (1.0 / np.maximum(deg, 1).astype(np.float32)).astype(np.float16)

    # per (core, block, half): edge lists
    per_core = []  # [core][block] -> (idxA, dlA, wA, idxB, dlB, wB)
    for c in range(NCORES):
        base = c * NPC
        blocks = []
        for bl in range(NBLK):
            lo = base + bl * BLK
            hi = min(base + (bl + 1) * BLK, base + NPC)
            e0, e1 = cum[lo], cum[hi]
            s = src[e0:e1]
            d_loc = (dst[e0:e1] - lo).astype(np.float16)  # 0..127 within block
            w = invdeg[dst[e0:e1]]
            selA = s < SPLIT
            blocks.append((
                s[selA], d_loc[selA], w[selA],
                s[~selA] - SPLIT, d_loc[~selA], w[~selA],
            ))
        per_core.append(blocks)

    # Per-core chunk counts per (block, half).  One program serves all 8
    # cores, so per-position caps are the max over cores; to keep that max
    # tight, each core processes its blocks in descending-chunk-count order
    # (sorted order statistics align across cores).  block_order[c][pos] is
    # the original block id handled at position pos.
    nchA_c = np.zeros((NCORES, NBLK), np.int64)
    nchB_c = np.zeros((NCORES, NBLK), np.int64)
    for c in range(NCORES):
        for bl in range(NBLK):
            blk = per_core[c][bl]
            nchA_c[c, bl] = max(-(-len(blk[0]) // BLK), 1)
            nchB_c[c, bl] = -(-len(blk[3]) // BLK)
    tot = nchA_c + nchB_c
    block_order = np.argsort(-tot, axis=1, kind="stable")  # [NCORES, NBLK]
    capA = np.zeros(NBLK, np.int64)
    capB = np.zeros(NBLK, np.int64)
    for pos in range(NBLK):
        capA[pos] = max(nchA_c[c, block_order[c, pos]] for c in range(NCORES))
        capB[pos] = max(nchB_c[c, block_order[c, pos]] for c in range(NCORES))

    nchA, nchB = int(capA.sum()), int(capB.sum())
    ncallsA = -(-nchA // CPC)
    ncallsB = -(-nchB // CPC)
    LA, LB = ncallsA * IDX_PER_CALL, ncallsB * IDX_PER_CALL
    CA, CB = ncallsA * CPC, ncallsB * CPC  # chunk columns incl. tail padding

    def wrap16(idx_stream, L):
        pad = np.zeros(L, np.int16)
        pad[: len(idx_stream)] = idx_stream.astype(np.int16)
        w = pad.reshape(L // 16, 16).T          # [16, L/16]
        return np.ascontiguousarray(np.tile(w, (8, 1)))  # [128, L/16]

    in_maps = []
    for c in range(NCORES):
        iA, iB = [], []
        dlA = np.full((128, CA), -1.0, np.float16)
        wA = np.zeros((128, CA), np.float16)
        dlB = np.full((128, CB), -1.0, np.float16)
        wB = np.zeros((128, CB), np.float16)
        jA = jB = 0
        for pos in range(NBLK):
            bl = int(block_order[c, pos])
            sA_, dA_, wA_, sB_, dB_, wB_ = per_core[c][bl]
            for half, (cap, s_, d_, w_) in enumerate((
                (capA[pos], sA_, dA_, wA_),
                (capB[pos], sB_, dB_, wB_),
            )):
                n = int(cap) * BLK
                si = np.zeros(n, np.int64)
                si[: len(s_)] = s_
                di = np.full(n, -1.0, np.float16)
                di[: len(d_)] = d_
                wi = np.zeros(n, np.float16)
                wi[: len(w_)] = w_
                if half == 0:
                    iA.append(si)
                    for k in range(int(cap)):
                        dlA[:, jA] = di[k * BLK:(k + 1) * BLK]
                        wA[:, jA] = wi[k * BLK:(k + 1) * BLK]
                        jA += 1
                else:
                    iB.append(si)
                    for k in range(int(cap)):
                        dlB[:, jB] = di[k * BLK:(k + 1) * BLK]
                        wB[:, jB] = wi[k * BLK:(k + 1) * BLK]
                        jB += 1
        assert jA == nchA and jB == nchB
        idxA = wrap16(np.concatenate(iA), LA)
        idxB = wrap16(np.concatenate(iB), LB)

        base = c * NPC
        # node permutation: position-order column i holds original local
        # node perm[i] (or padding where perm[i] < 0)
        perm = np.full(NPAD, -1, np.int64)
        for pos in range(NBLK):
            bl = int(block_order[c, pos])
            blkw = min(BLK, NPC - bl * BLK)
            perm[pos * BLK: pos * BLK + blkw] = np.arange(bl * BLK, bl * BLK + blkw)
        valid = perm >= 0
        xs = x16[base: base + NPC].astype(np.float32)
        mask = (deg[base: base + NPC] > 0).astype(np.float32)
        xT = np.zeros((D, NPAD), np.float16)
        xT[:, valid] = xs[perm[valid]].T
        xTm = np.zeros((D, NPAD), np.float16)
        xTm[:, valid] = (xs * mask[:, None])[perm[valid]].T

        in_maps.append({
            "xfull": x16,
            "xT": xT,
            "xTm": xTm,
            "WgT": np.ascontiguousarray(np.asarray(Wg, np.float32).T).astype(np.float16),
            "WsT": np.ascontiguousarray(np.asarray(Ws, np.float32).T).astype(np.float16),
            "WlT": np.ascontiguousarray(np.asarray(Wl, np.float32).T).astype(np.float16),
            "bias": np.asarray(b, np.float32).reshape(1, D).astype(np.float16),
            "iotab": np.broadcast_to(
                np.arange(BLK, dtype=np.float16)[None, :, None],
                (128, BLK, CPC)).copy(),
            "idxA": idxA,
            "idxB": idxB,
            "dlA": dlA, "wA": wA, "dlB": dlB, "wB": wB,
        })

    structure = (tuple(capA.tolist()), tuple(capB.tolist()), LA, LB)
    return structure, in_maps, block_order


def _build_program(structure):
    from concourse import bass, bacc, tile, mybir

    capA, capB, LA, LB = structure
    CA, CB = (LA // 128), (LB // 128)
    f16, f32, i16 = mybir.dt.float16, mybir.dt.float32, mybir.dt.int16

    nc = bacc.Bacc(
        "TRN2", target_bir_lowering=False, debug=False,
        num_devices=NCORES, num_swdge_queues=NQUEUES,
        dynamic_dma_scratch_size=32768,
    )

    xfull_d = nc.dram_tensor("xfull", [N, D], f16, kind="ExternalInput")
    xT_d = nc.dram_tensor("xT", [D, NPAD], f16, kind="ExternalInput")
    xTm_d = nc.dram_tensor("xTm", [D, NPAD], f16, kind="ExternalInput")
    WgT_d = nc.dram_tensor("WgT", [D, D], f16, kind="ExternalInput")
    WsT_d = nc.dram_tensor("WsT", [D, D], f16, kind="ExternalInput")
    WlT_d = nc.dram_tensor("WlT", [D, D], f16, kind="ExternalInput")
    bias_d = nc.dram_tensor("bias", [1, D], f16, kind="ExternalInput")
    iotab_d = nc.dram_tensor("iotab", [128, BLK, CPC], f16, kind="ExternalInput")
    idxA_d = nc.dram_tensor("idxA", [128, LA // 16], i16, kind="ExternalInput")
    idxB_d = nc.dram_tensor("idxB", [128, LB // 16], i16, kind="ExternalInput")
    dlA_d = nc.dram_tensor("dlA", [128, CA], f16, kind="ExternalInput")
    wA_d = nc.dram_tensor("wA", [128, CA], f16, kind="ExternalInput")
    dlB_d = nc.dram_tensor("dlB", [128, CB], f16, kind="ExternalInput")
    wB_d = nc.dram_tensor("wB", [128, CB], f16, kind="ExternalInput")
    out_d = nc.dram_tensor("outT", [D, NPAD], f32, kind="ExternalOutput")

    eq, mul_ = mybir.AluOpType.is_equal, mybir.AluOpType.mult
    add_, min_, max_ = mybir.AluOpType.add, mybir.AluOpType.min, mybir.AluOpType.max

    with tile.TileContext(nc) as tc:
        with (
            tc.tile_pool(name="res", bufs=1) as res,
            tc.tile_pool(name="gtA", bufs=5) as gtA,
            tc.tile_pool(name="gtB", bufs=5) as gtB,
            tc.tile_pool(name="ohA", bufs=5) as ohA,
            tc.tile_pool(name="ohB", bufs=5) as ohB,
            tc.tile_pool(name="nm", bufs=2) as nmp,
            tc.tile_pool(name="tmp", bufs=2) as tmp,
            tc.tile_pool(name="psA", bufs=3, space="PSUM") as psA,
            tc.tile_pool(name="psD", bufs=2, space="PSUM") as psD,
        ):
            # resident inputs
            xT_t = res.tile([D, NPAD], f16, tag="xT")
            xTm_t = res.tile([D, NPAD], f16, tag="xTm")
            WgT_t = res.tile([D, D], f16, tag="WgT")
            WsT_t = res.tile([D, D], f16, tag="WsT")
            WlT_t = res.tile([D, D], f16, tag="WlT")
            bias_t = res.tile([1, D], f16, tag="bias")
            iotab_t = res.tile([128, BLK, CPC], f16, tag="iotab")
            idxA_t = res.tile([128, LA // 16], i16, tag="idxA")
            idxB_t = res.tile([128, LB // 16], i16, tag="idxB")
            dlA_t = res.tile([128, CA], f16, tag="dlA")
            wA_t = res.tile([128, CA], f16, tag="wA")
            dlB_t = res.tile([128, CB], f16, tag="dlB")
            wB_t = res.tile([128, CB], f16, tag="wB")
            out_t = res.tile([D, NPAD], f32, tag="out")
            ones_t = res.tile([1, GRP], f16, tag="ones")

            # idx/dl/w first: the gather pipeline (the critical path) only
            # needs these; bulk xT/xTm loads follow.
            for t, d in ((idxA_t, idxA_d), (idxB_t, idxB_d),
                         (dlA_t, dlA_d), (wA_t, wA_d),
                         (dlB_t, dlB_d), (wB_t, wB_d),
                         (iotab_t, iotab_d), (WgT_t, WgT_d),
                         (WsT_t, WsT_d), (WlT_t, WlT_d), (bias_t, bias_d),
                         (xT_t, xT_d), (xTm_t, xTm_d)):
                nc.sync.dma_start(out=t[:], in_=d[:])
            nc.vector.memset(ones_t[:], 1.0)

            # gather-call bookkeeping: lazily emit gather + batched one-hot
            call_tiles = {}  # (half, k) -> (gather tile, onehot tile)
            qn = [0]

            def ensure_call(half, k):
                key = (half, k)
                if key in call_tiles:
                    return call_tiles[key]
                if half == 0:
                    gt = gtA.tile([128, CPC, D], f16, tag="gA")
                    oh = ohA.tile([128, BLK, CPC], f16, tag="oA")
                    idx_t, table = idxA_t, xfull_d[0:SPLIT, :]
                    dl_t, w_t = dlA_t, wA_t
                else:
                    gt = gtB.tile([128, CPC, D], f16, tag="gB")
                    oh = ohB.tile([128, BLK, CPC], f16, tag="oB")
                    idx_t, table = idxB_t, xfull_d[SPLIT:N, :]
                    dl_t, w_t = dlB_t, wB_t
                c0 = k * (IDX_PER_CALL // 16)
                nc.gpsimd.dma_gather(
                    gt[:], table, idx_t[:, c0:c0 + IDX_PER_CALL // 16],
                    IDX_PER_CALL, IDX_PER_CALL, D,
                    single_packet=False, queue_num=qn[0] % NQUEUES,
                )
                qn[0] += 1
                dl_v = dl_t[:, k * CPC:(k + 1) * CPC].unsqueeze(1) \
                    .broadcast_to([128, BLK, CPC])
                w_v = w_t[:, k * CPC:(k + 1) * CPC].unsqueeze(1) \
                    .broadcast_to([128, BLK, CPC])
                nc.vector.tensor_tensor(oh[:], iotab_t[:], dl_v, eq)
                nc.vector.tensor_tensor(oh[:], oh[:], w_v, mul_)
                call_tiles[key] = (gt, oh)
                return call_tiles[key]

            chunk_pos = [0, 0]  # next chunk index per half
            nm_t = None
            ps = None
            for bl in range(NBLK):
                g, sub = divmod(bl, GRP // BLK)
                gw = min(GRP, NPAD - g * GRP)
                if sub == 0:
                    ps = psA.tile([128, gw], f32, tag="agg")
                    nm_t = nmp.tile([128, gw], f16, tag="nm")
                col = sub * BLK
                nch_bl = int(capA[bl]) + int(capB[bl])
                done = 0
                for half, cap in ((0, capA[bl]), (1, capB[bl])):
                    for _ in range(int(cap)):
                        j = chunk_pos[half]
                        chunk_pos[half] += 1
                        gt, oh = ensure_call(half, j // CPC)
                        nc.tensor.matmul(
                            ps[:, col:col + BLK],
                            gt[:, j % CPC, :], oh[:, :, j % CPC],
                            start=(done == 0), stop=(done == nch_bl - 1),
                        )
                        done += 1

                if (sub + 1) * BLK == gw:  # group complete
                    nc.vector.tensor_copy(nm_t[:], ps[:])
                    g0 = g * GRP
                    pd = psD.tile([128, gw], f32, tag="dense")
                    nc.tensor.matmul(pd[:], WgT_t[:], xT_t[:, g0:g0 + gw],
                                     start=True, stop=False)
                    nc.tensor.matmul(pd[:], WsT_t[:], xTm_t[:, g0:g0 + gw],
                                     start=False, stop=False)
                    nc.tensor.matmul(pd[:], WlT_t[:], nm_t[:],
                                     start=False, stop=False)
                    nc.tensor.matmul(pd[:], bias_t[:], ones_t[:, :gw],
                                     start=False, stop=True)
                    xm = tmp.tile([128, gw], f32, tag="xm")
                    nc.vector.tensor_scalar(xm[:], pd[:], 0.0, None, min_)
                    ex = tmp.tile([128, gw], f32, tag="ex")
                    nc.scalar.activation(ex[:], xm[:],
                                         mybir.ActivationFunctionType.Exp)
                    r1 = tmp.tile([128, gw], f32, tag="r1")
                    nc.vector.tensor_scalar(r1[:], pd[:], 0.0, -1.0, max_, add_)
                    nc.vector.tensor_tensor(out_t[:, g0:g0 + gw], r1[:], ex[:], add_)
                    nc.sync.dma_start(out=out_d[:, g0:g0 + gw],
                                      in_=out_t[:, g0:g0 + gw])

    nc.compile()
    return nc


def kernel(x, Wg, Wl, Ws, b, src, dst, deg, _trace=False):
    from concourse.bass_utils import run_bass_kernel_spmd

    structure, in_maps, block_order = _build_host_data(x, Wg, Wl, Ws, b, src, dst, deg)
    if structure not in _cache:
        _cache[structure] = _build_program(structure)
    nc = _cache[structure]

    kwargs = {}
    if _trace:
        import types, importlib.util
        if importlib.util.find_spec("antenv.axon_hooks") is None:
            mod = types.ModuleType("antenv.axon_hooks")
            mod._hook = None
            mod.set_axon_ntff_profile_hook = lambda h: setattr(mod, "_hook", h)
            mod.get_axon_ntff_profile_hook = lambda: mod._hook
            sys.modules["antenv.axon_hooks"] = mod
            import antenv
            antenv.axon_hooks = mod
            from trn_agent_boot.trn_boot import _ntff_profile_via_ctypes
            mod.set_axon_ntff_profile_hook(
                _ntff_profile_via_ctypes("/opt/axon/libaxon_pjrt.so"))
        from concourse import bass_utils as _bu
        _bu.upload_artifacts = lambda tmpdir: tmpdir
        kwargs["trace"] = True

    res = run_bass_kernel_spmd(nc, in_maps, list(range(NCORES)), **kwargs)

    out = np.empty((N, D), np.float32)
    for c in range(NCORES):
        oT = res.results[c]["outT"]
        for pos in range(NBLK):
            bl = int(block_order[c, pos])
            blkw = min(BLK, NPC - bl * BLK)
            out[c * NPC + bl * BLK: c * NPC + bl * BLK + blkw] = \
                oT[:, pos * BLK: pos * BLK + blkw].T
    if _trace:
        kernel.last_exec_time_ns = res.exec_time_ns
    return out


# revision 10
# speedup vs baseline: 1.0511x; 1.0511x over previous
"""GNN message-passing layer (nn_DEMOWeightLayer) on 8 Trainium2 NeuronCores.

Reference computation (per node i):
    out = elu(x @ Wg.T + [deg>0] * (mean_{src->i} x[src] @ Wl.T + x @ Ws.T) + b)

Sharding: nodes (and their incoming edges, since dst is sorted) are split
contiguously across 8 cores; x is replicated to every core's HBM so no
cross-core exchange is needed.  Per core, everything is computed in a
feature-on-partitions (transposed) layout:

  - x[src] rows (f16, 256B) are fetched with dma_gather (SWDGE, 4 queues
    for parallel Q7 descriptor generation).  src indices are int16, so
    edges are split into two streams by src < 32768 (table A = x[:32768],
    B = x[32768:]).
  - segment-sum becomes a one-hot matmul per 128-edge chunk:
        psum[feat, 128 nodes] += chunk[128e, feat].T @ onehot[128e, 128n]
    with onehot[e, n] = (dst_local[e] == n) * (1/deg[dst[e]]).  One-hots
    for all 16 chunks of a gather call are built in two batched DVE ops
    using stride-0 broadcast access patterns.
  - edges of each (128-node block, half) are padded to whole chunks with
    (idx=0, dl=-1, w=0); chunk counts per (block, half) are the max over
    the 8 cores so one program serves all cores (SPMD).
  - dense branch per 512-node group: Wg@xT + Ws@xTm + Wl@nm + b*1 in PSUM,
    then ELU = (max(y,0)-1) + exp(min(y,0)) via DVE+ACT.
"""

import sys
import numpy as np

sys.path.insert(0, "/opt/trn_rl_repo")

N = 50000
D = 128
NCORES = 8
NPC = N // NCORES            # 6250 nodes per core
BLK = 128
NBLK = (NPC + BLK - 1) // BLK  # 49 blocks (6272 padded nodes)
NPAD = NBLK * BLK
SPLIT = 32768                # int16 index limit
GRP = 512                    # dense-phase node group (one PSUM bank)
IDX_PER_CALL = 2048          # per dma_gather call (16 chunks)
CPC = IDX_PER_CALL // 128    # chunks per call
NQUEUES = 4

_cache = {}


def _build_host_data(x, Wg, Wl, Ws, b, src, dst, deg):
    """Shard + pad the edge streams; returns (structure, per-core in_maps)."""
    x16 = np.asarray(x, np.float32).astype(np.float16)
    deg = np.asarray(deg, np.int64)
    src = np.asarray(src, np.int64)
    dst = np.asarray(dst, np.int64)

    cum = np.concatenate([[0], np.cumsum(deg)])
    invdeg = (1.0 / np.maximum(deg, 1).astype(np.float32)).astype(np.float16)

    # per (core, block, half): edge lists
    per_core = []  # [core][block] -> (idxA, dlA, wA, idxB, dlB, wB)
    for c in range(NCORES):
        base = c * NPC
        blocks = []
        for bl in range(NBLK):
            lo = base + bl * BLK
            hi = min(base + (bl + 1) * BLK, base + NPC)
            e0, e1 = cum[lo], cum[hi]
            s = src[e0:e1]
            d_loc = (dst[e0:e1] - lo).astype(np.float16)  # 0..127 within block
            w = invdeg[dst[e0:e1]]
            selA = s < SPLIT
            blocks.append((
                s[selA], d_loc[selA], w[selA],
                s[~selA] - SPLIT, d_loc[~selA], w[~selA],
            ))
        per_core.append(blocks)

    # Per-core chunk counts per (block, half).  One program serves all 8
    # cores, so per-position caps are the max over cores; to keep that max
    # tight, each core processes its blocks in descending-chunk-count order
    # (sorted order statistics align across cores).  block_order[c][pos] is
    # the original block id handled at position pos.
    nchA_c = np.zeros((NCORES, NBLK), np.int64)
    nchB_c = np.zeros((NCORES, NBLK), np.int64)
    for c in range(NCORES):
        for bl in range(NBLK):
            blk = per_core[c][bl]
            nchA_c[c, bl] = max(-(-len(blk[0]) // BLK), 1)
            nchB_c[c, bl] = -(-len(blk[3]) // BLK)
    tot = nchA_c + nchB_c
    block_order = np.argsort(-tot, axis=1, kind="stable")  # [NCORES, NBLK]
    capA = np.zeros(NBLK, np.int64)
    capB = np.zeros(NBLK, np.int64)
    for pos in range(NBLK):
        capA[pos] = max(nchA_c[c, block_order[c, pos]] for c in range(NCORES))
        capB[pos] = max(nchB_c[c, block_order[c, pos]] for c in range(NCORES))

    nchA, nchB = int(capA.sum()), int(capB.sum())
    ncallsA = -(-nchA // CPC)
    ncallsB = -(-nchB // CPC)
    LA, LB = ncallsA * IDX_PER_CALL, ncallsB * IDX_PER_CALL
    CA, CB = ncallsA * CPC, ncallsB * CPC  # chunk columns incl. tail padding

    def wrap16(idx_stream, L):
        pad = np.zeros(L, np.int16)
        pad[: len(idx_stream)] = idx_stream.astype(np.int16)
        w = pad.reshape(L // 16, 16).T          # [16, L/16]
        return np.ascontiguousarray(np.tile(w, (8, 1)))  # [128, L/16]

    in_maps = []
    for c in range(NCORES):
        iA, iB = [], []
        dlA = np.full((128, CA), -1.0, np.float16)
        wA = np.zeros((128, CA), np.float16)
        dlB = np.full((128, CB), -1.0, np.float16)
        wB = np.zeros((128, CB), np.float16)
        jA = jB = 0
        for pos in range(NBLK):
            bl = int(block_order[c, pos])
            sA_, dA_, wA_, sB_, dB_, wB_ = per_core[c][bl]
            for half, (cap, s_, d_, w_) in enumerate((
                (capA[pos], sA_, dA_, wA_),
                (capB[pos], sB_, dB_, wB_),
            )):
                n = int(cap) * BLK
                si = np.zeros(n, np.int64)
                si[: len(s_)] = s_
                di = np.full(n, -1.0, np.float16)
                di[: len(d_)] = d_
                wi = np.zeros(n, np.float16)
                wi[: len(w_)] = w_
                if half == 0:
                    iA.append(si)
                    for k in range(int(cap)):
                        dlA[:, jA] = di[k * BLK:(k + 1) * BLK]
                        wA[:, jA] = wi[k * BLK:(k + 1) * BLK]
                        jA += 1
                else:
                    iB.append(si)
                    for k in range(int(cap)):
                        dlB[:, jB] = di[k * BLK:(k + 1) * BLK]
                        wB[:, jB] = wi[k * BLK:(k + 1) * BLK]
                        jB += 1
        assert jA == nchA and jB == nchB
        idxA = wrap16(np.concatenate(iA), LA)
        idxB = wrap16(np.concatenate(iB), LB)

        base = c * NPC
        # node permutation: position-order column i holds original local
        # node perm[i] (or padding where perm[i] < 0)
        perm = np.full(NPAD, -1, np.int64)
        for pos in range(NBLK):
            bl = int(block_order[c, pos])
            blkw = min(BLK, NPC - bl * BLK)
            perm[pos * BLK: pos * BLK + blkw] = np.arange(bl * BLK, bl * BLK + blkw)
        valid = perm >= 0
        xs = x16[base: base + NPC].astype(np.float32)
        mask = (deg[base: base + NPC] > 0).astype(np.float32)
        xT = np.zeros((D, NPAD), np.float16)
        xT[:, valid] = xs[perm[valid]].T
        xTm = np.zeros((D, NPAD), np.float16)
        xTm[:, valid] = (xs * mask[:, None])[perm[valid]].T

        in_maps.append({
            "xfull": x16,
            "xT": xT,
            "xTm": xTm,
            "WgT": np.ascontiguousarray(np.asarray(Wg, np.float32).T).astype(np.float16),
            "WsT": np.ascontiguousarray(np.asarray(Ws, np.float32).T).astype(np.float16),
            "WlT": np.ascontiguousarray(np.asarray(Wl, np.float32).T).astype(np.float16),
            "bias": np.asarray(b, np.float32).reshape(1, D).astype(np.float16),
            "iotab": np.broadcast_to(
                np.arange(BLK, dtype=np.float16)[None, :, None],
                (128, BLK, CPC)).copy(),
            "idxA": idxA,
            "idxB": idxB,
            "dlA": dlA, "wA": wA, "dlB": dlB, "wB": wB,
        })

    structure = (tuple(capA.tolist()), tuple(capB.tolist()), LA, LB)
    return structure, in_maps, block_order


def _build_program(structure):
    from concourse import bass, bacc, tile, mybir

    capA, capB, LA, LB = structure
    CA, CB = (LA // 128), (LB // 128)
    f16, f32, i16 = mybir.dt.float16, mybir.dt.float32, mybir.dt.int16

    nc = bacc.Bacc(
        "TRN2", target_bir_lowering=False, debug=False,
        num_devices=NCORES, num_swdge_queues=NQUEUES,
        dynamic_dma_scratch_size=32768,
    )

    xfull_d = nc.dram_tensor("xfull", [N, D], f16, kind="ExternalInput")
    xT_d = nc.dram_tensor("xT", [D, NPAD], f16, kind="ExternalInput")
    xTm_d = nc.dram_tensor("xTm", [D, NPAD], f16, kind="ExternalInput")
    WgT_d = nc.dram_tensor("WgT", [D, D], f16, kind="ExternalInput")
    WsT_d = nc.dram_tensor("WsT", [D, D], f16, kind="ExternalInput")
    WlT_d = nc.dram_tensor("WlT", [D, D], f16, kind="ExternalInput")
    bias_d = nc.dram_tensor("bias", [1, D], f16, kind="ExternalInput")
    iotab_d = nc.dram_tensor("iotab", [128, BLK, CPC], f16, kind="ExternalInput")
    idxA_d = nc.dram_tensor("idxA", [128, LA // 16], i16, kind="ExternalInput")
    idxB_d = nc.dram_tensor("idxB", [128, LB // 16], i16, kind="ExternalInput")
    dlA_d = nc.dram_tensor("dlA", [128, CA], f16, kind="ExternalInput")
    wA_d = nc.dram_tensor("wA", [128, CA], f16, kind="ExternalInput")
    dlB_d = nc.dram_tensor("dlB", [128, CB], f16, kind="ExternalInput")
    wB_d = nc.dram_tensor("wB", [128, CB], f16, kind="ExternalInput")
    out_d = nc.dram_tensor("outT", [D, NPAD], f32, kind="ExternalOutput")

    eq, mul_ = mybir.AluOpType.is_equal, mybir.AluOpType.mult
    add_, min_, max_ = mybir.AluOpType.add, mybir.AluOpType.min, mybir.AluOpType.max

    with tile.TileContext(nc) as tc:
        with (
            tc.tile_pool(name="res", bufs=1) as res,
            tc.tile_pool(name="gtA", bufs=5) as gtA,
            tc.tile_pool(name="gtB", bufs=5) as gtB,
            tc.tile_pool(name="ohA", bufs=5) as ohA,
            tc.tile_pool(name="ohB", bufs=5) as ohB,
            tc.tile_pool(name="nm", bufs=2) as nmp,
            tc.tile_pool(name="tmp", bufs=2) as tmp,
            tc.tile_pool(name="psA", bufs=3, space="PSUM") as psA,
            tc.tile_pool(name="psD", bufs=2, space="PSUM") as psD,
        ):
            # resident inputs
            xT_t = res.tile([D, NPAD], f16, tag="xT")
            xTm_t = res.tile([D, NPAD], f16, tag="xTm")
            WgT_t = res.tile([D, D], f16, tag="WgT")
            WsT_t = res.tile([D, D], f16, tag="WsT")
            WlT_t = res.tile([D, D], f16, tag="WlT")
            bias_t = res.tile([1, D], f16, tag="bias")
            iotab_t = res.tile([128, BLK, CPC], f16, tag="iotab")
            idxA_t = res.tile([128, LA // 16], i16, tag="idxA")
            idxB_t = res.tile([128, LB // 16], i16, tag="idxB")
            dlA_t = res.tile([128, CA], f16, tag="dlA")
            wA_t = res.tile([128, CA], f16, tag="wA")
            dlB_t = res.tile([128, CB], f16, tag="dlB")
            wB_t = res.tile([128, CB], f16, tag="wB")
            out_t = res.tile([D, NPAD], f32, tag="out")
            ones_t = res.tile([1, GRP], f16, tag="ones")

            # idx/dl/w first: the gather pipeline (the critical path) only
            # needs these; bulk xT/xTm loads follow.
            for t, d in ((idxA_t, idxA_d), (idxB_t, idxB_d),
                         (dlA_t, dlA_d), (wA_t, wA_d),
                         (dlB_t, dlB_d), (wB_t, wB_d),
                         (iotab_t, iotab_d), (WgT_t, WgT_d),
                         (WsT_t, WsT_d), (WlT_t, WlT_d), (bias_t, bias_d),
                         (xT_t, xT_d), (xTm_t, xTm_d)):
                nc.sync.dma_start(out=t[:], in_=d[:])
            nc.vector.memset(ones_t[:], 1.0)

            # gather-call bookkeeping: lazily emit gather + batched one-hot
            call_tiles = {}  # (half, k) -> (gather tile, onehot tile)
            qn = [0]

            def ensure_call(half, k):
                key = (half, k)
                if key in call_tiles:
                    return call_tiles[key]
                if half == 0:
                    gt = gtA.tile([128, CPC, D], f16, tag="gA")
                    oh = ohA.tile([128, BLK, CPC], f16, tag="oA")
                    idx_t, table = idxA_t, xfull_d[0:SPLIT, :]
                    dl_t, w_t = dlA_t, wA_t
                else:
                    gt = gtB.tile([128, CPC, D], f16, tag="gB")
                    oh = ohB.tile([128, BLK, CPC], f16, tag="oB")
                    idx_t, table = idxB_t, xfull_d[SPLIT:N, :]
                    dl_t, w_t = dlB_t, wB_t
                c0 = k * (IDX_PER_CALL // 16)
                nc.gpsimd.dma_gather(
                    gt[:], table, idx_t[:, c0:c0 + IDX_PER_CALL // 16],
                    IDX_PER_CALL, IDX_PER_CALL, D,
                    single_packet=False, queue_num=qn[0] % NQUEUES,
                )
                qn[0] += 1
                dl_v = dl_t[:, k * CPC:(k + 1) * CPC].unsqueeze(1) \
                    .broadcast_to([128, BLK, CPC])
                w_v = w_t[:, k * CPC:(k + 1) * CPC].unsqueeze(1) \
                    .broadcast_to([128, BLK, CPC])
                nc.vector.tensor_tensor(oh[:], iotab_t[:], dl_v, eq)
                nc.vector.tensor_tensor(oh[:], oh[:], w_v, mul_)
                call_tiles[key] = (gt, oh)
                return call_tiles[key]

            chunk_pos = [0, 0]  # next chunk index per half
            nm_t = None
            ps = None
            for bl in range(NBLK):
                g, sub = divmod(bl, GRP // BLK)
                gw = min(GRP, NPAD - g * GRP)
                if sub == 0:
                    ps = psA.tile([128, gw], f32, tag="agg")
                    nm_t = nmp.tile([128, gw], f16, tag="nm")
                col = sub * BLK
                nch_bl = int(capA[bl]) + int(capB[bl])
                done = 0
                for half, cap in ((0, capA[bl]), (1, capB[bl])):
                    for _ in range(int(cap)):
                        j = chunk_pos[half]
                        chunk_pos[half] += 1
                        gt, oh = ensure_call(half, j // CPC)
                        nc.tensor.matmul(
                            ps[:, col:col + BLK],
                            gt[:, j % CPC, :], oh[:, :, j % CPC],
                            start=(done == 0), stop=(done == nch_bl - 1),
                        )
                        done += 1

                if (sub + 1) * BLK == gw:  # group complete
                    nc.vector.tensor_copy(nm_t[:], ps[:])
                    g0 = g * GRP
                    pd = psD.tile([128, gw], f32, tag="dense")
                    nc.tensor.matmul(pd[:], WgT_t[:], xT_t[:, g0:g0 + gw],
                                     start=True, stop=False)
                    nc.tensor.matmul(pd[:], WsT_t[:], xTm_t[:, g0:g0 + gw],
                                     start=False, stop=False)
                    nc.tensor.matmul(pd[:], WlT_t[:], nm_t[:],
                                     start=False, stop=False)
                    nc.tensor.matmul(pd[:], bias_t[:], ones_t[:, :gw],
                                     start=False, stop=True)
                    xm = tmp.tile([128, gw], f32, tag="xm")
                    nc.vector.tensor_scalar(xm[:], pd[:], 0.0, None, min_)
                    ex = tmp.tile([128, gw], f32, tag="ex")
                    nc.scalar.activation(ex[:], xm[:],
                                         mybir.ActivationFunctionType.Exp)
                    r1 = tmp.tile([128, gw], f32, tag="r1")
                    nc.vector.tensor_scalar(r1[:], pd[:], 0.0, -1.0, max_, add_)
                    nc.vector.tensor_tensor(out_t[:, g0:g0 + gw], r1[:], ex[:], add_)

            nc.sync.dma_start(out=out_d[:], in_=out_t[:])

    nc.compile()
    return nc


def kernel(x, Wg, Wl, Ws, b, src, dst, deg, _trace=False):
    from concourse.bass_utils import run_bass_kernel_spmd

    structure, in_maps, block_order = _build_host_data(x, Wg, Wl, Ws, b, src, dst, deg)
    if structure not in _cache:
        _cache[structure] = _build_program(structure)
    nc = _cache[structure]

    kwargs = {}
    if _trace:
        import types, importlib.util
        if importlib.util.find_spec("antenv.axon_hooks") is None:
            mod = types.ModuleType("antenv.axon_hooks")
            mod._hook = None
            mod.set_axon_ntff_profile_hook = lambda h: setattr(mod, "_hook", h)
            mod.get_axon_ntff_profile_hook = lambda: mod._hook
            sys.modules["antenv.axon_hooks"] = mod
            import antenv
            antenv.axon_hooks = mod
            from trn_agent_boot.trn_boot import _ntff_profile_via_ctypes
            mod.set_axon_ntff_profile_hook(
                _ntff_profile_via_ctypes("/opt/axon/libaxon_pjrt.so"))
        from concourse import bass_utils as _bu
        _bu.upload_artifacts = lambda tmpdir: tmpdir
        kwargs["trace"] = True

    res = run_bass_kernel_spmd(nc, in_maps, list(range(NCORES)), **kwargs)

    out = np.empty((N, D), np.float32)
    for c in range(NCORES):
        oT = res.results[c]["outT"]
        for pos in range(NBLK):
            bl = int(block_order[c, pos])
            blkw = min(BLK, NPC - bl * BLK)
            out[c * NPC + bl * BLK: c * NPC + bl * BLK + blkw] = \
                oT[:, pos * BLK: pos * BLK + blkw].T
    if _trace:
        kernel.last_exec_time_ns = res.exec_time_ns
    return out


# revision 11
# speedup vs baseline: 1.0571x; 1.0057x over previous
"""GNN message-passing layer (nn_DEMOWeightLayer) on 8 Trainium2 NeuronCores.

Reference computation (per node i):
    out = elu(x @ Wg.T + [deg>0] * (mean_{src->i} x[src] @ Wl.T + x @ Ws.T) + b)

Sharding: nodes (and their incoming edges, since dst is sorted) are split
contiguously across 8 cores; x is replicated to every core's HBM so no
cross-core exchange is needed.  Per core, everything is computed in a
feature-on-partitions (transposed) layout:

  - x[src] rows (f16, 256B) are fetched with dma_gather (SWDGE, 4 queues
    for parallel Q7 descriptor generation).  src indices are int16, so
    edges are split into two streams by src < 32768 (table A = x[:32768],
    B = x[32768:]).
  - segment-sum becomes a one-hot matmul per 128-edge chunk:
        psum[feat, 128 nodes] += chunk[128e, feat].T @ onehot[128e, 128n]
    with onehot[e, n] = (dst_local[e] == n) * (1/deg[dst[e]]).  One-hots
    for all 16 chunks of a gather call are built in two batched DVE ops
    using stride-0 broadcast access patterns.
  - edges of each (128-node block, half) are padded to whole chunks with
    (idx=0, dl=-1, w=0); chunk counts per (block, half) are the max over
    the 8 cores so one program serves all cores (SPMD).
  - dense branch per 512-node group: Wg@xT + Ws@xTm + Wl@nm + b*1 in PSUM,
    then ELU = (max(y,0)-1) + exp(min(y,0)) via DVE+ACT.
"""

import sys
import numpy as np

sys.path.insert(0, "/opt/trn_rl_repo")

N = 50000
D = 128
NCORES = 8
NPC = N // NCORES            # 6250 nodes per core
BLK = 128
NBLK = (NPC + BLK - 1) // BLK  # 49 blocks (6272 padded nodes)
NPAD = NBLK * BLK
SPLIT = 32768                # int16 index limit
GRP = 512                    # dense-phase node group (one PSUM bank)
IDX_PER_CALL = 2048          # per dma_gather call (16 chunks)
CPC = IDX_PER_CALL // 128    # chunks per call
NQUEUES = 4

_cache = {}


def _build_host_data(x, Wg, Wl, Ws, b, src, dst, deg):
    """Shard + pad the edge streams; returns (structure, per-core in_maps)."""
    x16 = np.asarray(x, np.float32).astype(np.float16)
    deg = np.asarray(deg, np.int64)
    src = np.asarray(src, np.int64)
    dst = np.asarray(dst, np.int64)

    cum = np.concatenate([[0], np.cumsum(deg)])
    invdeg = (1.0 / np.maximum(deg, 1).astype(np.float32)).astype(np.float16)

    # per (core, block, half): edge lists
    per_core = []  # [core][block] -> (idxA, dlA, wA, idxB, dlB, wB)
    for c in range(NCORES):
        base = c * NPC
        blocks = []
        for bl in range(NBLK):
            lo = base + bl * BLK
            hi = min(base + (bl + 1) * BLK, base + NPC)
            e0, e1 = cum[lo], cum[hi]
            s = src[e0:e1]
            d_loc = (dst[e0:e1] - lo).astype(np.float16)  # 0..127 within block
            w = invdeg[dst[e0:e1]]
            selA = s < SPLIT
            blocks.append((
                s[selA], d_loc[selA], w[selA],
                s[~selA] - SPLIT, d_loc[~selA], w[~selA],
            ))
        per_core.append(blocks)

    # Per-core chunk counts per (block, half).  One program serves all 8
    # cores, so per-position caps are the max over cores; to keep that max
    # tight, each core processes its blocks in descending-chunk-count order
    # (sorted order statistics align across cores).  block_order[c][pos] is
    # the original block id handled at position pos.
    nchA_c = np.zeros((NCORES, NBLK), np.int64)
    nchB_c = np.zeros((NCORES, NBLK), np.int64)
    for c in range(NCORES):
        for bl in range(NBLK):
            blk = per_core[c][bl]
            nchA_c[c, bl] = max(-(-len(blk[0]) // BLK), 1)
            nchB_c[c, bl] = -(-len(blk[3]) // BLK)
    tot = nchA_c + nchB_c
    block_order = np.argsort(-tot, axis=1, kind="stable")  # [NCORES, NBLK]
    capA = np.zeros(NBLK, np.int64)
    capB = np.zeros(NBLK, np.int64)
    for pos in range(NBLK):
        capA[pos] = max(nchA_c[c, block_order[c, pos]] for c in range(NCORES))
        capB[pos] = max(nchB_c[c, block_order[c, pos]] for c in range(NCORES))

    nchA, nchB = int(capA.sum()), int(capB.sum())
    ncallsA = -(-nchA // CPC)
    ncallsB = -(-nchB // CPC)
    LA, LB = ncallsA * IDX_PER_CALL, ncallsB * IDX_PER_CALL
    CA, CB = ncallsA * CPC, ncallsB * CPC  # chunk columns incl. tail padding

    def wrap16(idx_stream, L):
        pad = np.zeros(L, np.int16)
        pad[: len(idx_stream)] = idx_stream.astype(np.int16)
        w = pad.reshape(L // 16, 16).T          # [16, L/16]
        return np.ascontiguousarray(np.tile(w, (8, 1)))  # [128, L/16]

    in_maps = []
    for c in range(NCORES):
        iA, iB = [], []
        dlA = np.full((128, CA), -1.0, np.float16)
        wA = np.zeros((128, CA), np.float16)
        dlB = np.full((128, CB), -1.0, np.float16)
        wB = np.zeros((128, CB), np.float16)
        jA = jB = 0
        for pos in range(NBLK):
            bl = int(block_order[c, pos])
            sA_, dA_, wA_, sB_, dB_, wB_ = per_core[c][bl]
            for half, (cap, s_, d_, w_) in enumerate((
                (capA[pos], sA_, dA_, wA_),
                (capB[pos], sB_, dB_, wB_),
            )):
                n = int(cap) * BLK
                si = np.zeros(n, np.int64)
                si[: len(s_)] = s_
                di = np.full(n, -1.0, np.float16)
                di[: len(d_)] = d_
                wi = np.zeros(n, np.float16)
                wi[: len(w_)] = w_
                if half == 0:
                    iA.append(si)
                    for k in range(int(cap)):
                        dlA[:, jA] = di[k * BLK:(k + 1) * BLK]
                        wA[:, jA] = wi[k * BLK:(k + 1) * BLK]
                        jA += 1
                else:
                    iB.append(si)
                    for k in range(int(cap)):
                        dlB[:, jB] = di[k * BLK:(k + 1) * BLK]
                        wB[:, jB] = wi[k * BLK:(k + 1) * BLK]
                        jB += 1
        assert jA == nchA and jB == nchB
        idxA = wrap16(np.concatenate(iA), LA)
        idxB = wrap16(np.concatenate(iB), LB)

        base = c * NPC
        # node permutation: position-order column i holds original local
        # node perm[i] (or padding where perm[i] < 0)
        perm = np.full(NPAD, -1, np.int64)
        for pos in range(NBLK):
            bl = int(block_order[c, pos])
            blkw = min(BLK, NPC - bl * BLK)
            perm[pos * BLK: pos * BLK + blkw] = np.arange(bl * BLK, bl * BLK + blkw)
        valid = perm >= 0
        xs = x16[base: base + NPC].astype(np.float32)
        mask = (deg[base: base + NPC] > 0).astype(np.float32)
        xT = np.zeros((D, NPAD), np.float16)
        xT[:, valid] = xs[perm[valid]].T
        xTm = np.zeros((D, NPAD), np.float16)
        xTm[:, valid] = (xs * mask[:, None])[perm[valid]].T

        in_maps.append({
            "xfull": x16,
            "xT": xT,
            "xTm": xTm,
            "WgT": np.ascontiguousarray(np.asarray(Wg, np.float32).T).astype(np.float16),
            "WsT": np.ascontiguousarray(np.asarray(Ws, np.float32).T).astype(np.float16),
            "WlT": np.ascontiguousarray(np.asarray(Wl, np.float32).T).astype(np.float16),
            "bias": np.asarray(b, np.float32).reshape(1, D).astype(np.float16),
            "iotab": np.broadcast_to(
                np.arange(BLK, dtype=np.float16)[None, :, None],
                (128, BLK, CPC)).copy(),
            "idxA": idxA,
            "idxB": idxB,
            "dlA": dlA, "wA": wA, "dlB": dlB, "wB": wB,
        })

    structure = (tuple(capA.tolist()), tuple(capB.tolist()), LA, LB)
    return structure, in_maps, block_order


def _build_program(structure):
    from concourse import bass, bacc, tile, mybir

    capA, capB, LA, LB = structure
    CA, CB = (LA // 128), (LB // 128)
    f16, f32, i16 = mybir.dt.float16, mybir.dt.float32, mybir.dt.int16

    nc = bacc.Bacc(
        "TRN2", target_bir_lowering=False, debug=False,
        num_devices=NCORES, num_swdge_queues=NQUEUES,
        dynamic_dma_scratch_size=32768,
    )

    xfull_d = nc.dram_tensor("xfull", [N, D], f16, kind="ExternalInput")
    xT_d = nc.dram_tensor("xT", [D, NPAD], f16, kind="ExternalInput")
    xTm_d = nc.dram_tensor("xTm", [D, NPAD], f16, kind="ExternalInput")
    WgT_d = nc.dram_tensor("WgT", [D, D], f16, kind="ExternalInput")
    WsT_d = nc.dram_tensor("WsT", [D, D], f16, kind="ExternalInput")
    WlT_d = nc.dram_tensor("WlT", [D, D], f16, kind="ExternalInput")
    bias_d = nc.dram_tensor("bias", [1, D], f16, kind="ExternalInput")
    iotab_d = nc.dram_tensor("iotab", [128, BLK, CPC], f16, kind="ExternalInput")
    idxA_d = nc.dram_tensor("idxA", [128, LA // 16], i16, kind="ExternalInput")
    idxB_d = nc.dram_tensor("idxB", [128, LB // 16], i16, kind="ExternalInput")
    dlA_d = nc.dram_tensor("dlA", [128, CA], f16, kind="ExternalInput")
    wA_d = nc.dram_tensor("wA", [128, CA], f16, kind="ExternalInput")
    dlB_d = nc.dram_tensor("dlB", [128, CB], f16, kind="ExternalInput")
    wB_d = nc.dram_tensor("wB", [128, CB], f16, kind="ExternalInput")
    out_d = nc.dram_tensor("outT", [D, NPAD], f32, kind="ExternalOutput")

    eq, mul_ = mybir.AluOpType.is_equal, mybir.AluOpType.mult
    add_, min_, max_ = mybir.AluOpType.add, mybir.AluOpType.min, mybir.AluOpType.max

    with tile.TileContext(nc) as tc:
        with (
            tc.tile_pool(name="res", bufs=1) as res,
            tc.tile_pool(name="gtA", bufs=5) as gtA,
            tc.tile_pool(name="gtB", bufs=5) as gtB,
            tc.tile_pool(name="ohA", bufs=5) as ohA,
            tc.tile_pool(name="ohB", bufs=5) as ohB,
            tc.tile_pool(name="nm", bufs=2) as nmp,
            tc.tile_pool(name="tmp", bufs=2) as tmp,
            tc.tile_pool(name="psA", bufs=3, space="PSUM") as psA,
            tc.tile_pool(name="psD", bufs=2, space="PSUM") as psD,
        ):
            # resident inputs
            xT_t = res.tile([D, NPAD], f16, tag="xT")
            xTm_t = res.tile([D, NPAD], f16, tag="xTm")
            WgT_t = res.tile([D, D], f16, tag="WgT")
            WsT_t = res.tile([D, D], f16, tag="WsT")
            WlT_t = res.tile([D, D], f16, tag="WlT")
            bias_t = res.tile([1, D], f16, tag="bias")
            iotab_t = res.tile([128, BLK, CPC], f16, tag="iotab")
            idxA_t = res.tile([128, LA // 16], i16, tag="idxA")
            idxB_t = res.tile([128, LB // 16], i16, tag="idxB")
            dlA_t = res.tile([128, CA], f16, tag="dlA")
            wA_t = res.tile([128, CA], f16, tag="wA")
            dlB_t = res.tile([128, CB], f16, tag="dlB")
            wB_t = res.tile([128, CB], f16, tag="wB")
            out_t = res.tile([D, NPAD], f32, tag="out")
            ones_t = res.tile([1, GRP], f16, tag="ones")

            # idx/dl/w first: the gather pipeline (the critical path) only
            # needs these; bulk xT/xTm loads follow.
            for t, d in ((idxA_t, idxA_d), (idxB_t, idxB_d),
                         (dlA_t, dlA_d), (wA_t, wA_d),
                         (dlB_t, dlB_d), (wB_t, wB_d),
                         (iotab_t, iotab_d), (WgT_t, WgT_d),
                         (WsT_t, WsT_d), (WlT_t, WlT_d), (bias_t, bias_d),
                         (xT_t, xT_d), (xTm_t, xTm_d)):
                nc.sync.dma_start(out=t[:], in_=d[:])
            nc.vector.memset(ones_t[:], 1.0)

            # gather-call bookkeeping: lazily emit gather + batched one-hot
            call_tiles = {}  # (half, k) -> (gather tile, onehot tile)
            qn = [0]

            def ensure_call(half, k):
                key = (half, k)
                if key in call_tiles:
                    return call_tiles[key]
                if half == 0:
                    gt = gtA.tile([128, CPC, D], f16, tag="gA")
                    oh = ohA.tile([128, BLK, CPC], f16, tag="oA")
                    idx_t, table = idxA_t, xfull_d[0:SPLIT, :]
                    dl_t, w_t = dlA_t, wA_t
                else:
                    gt = gtB.tile([128, CPC, D], f16, tag="gB")
                    oh = ohB.tile([128, BLK, CPC], f16, tag="oB")
                    idx_t, table = idxB_t, xfull_d[SPLIT:N, :]
                    dl_t, w_t = dlB_t, wB_t
                c0 = k * (IDX_PER_CALL // 16)
                nc.gpsimd.dma_gather(
                    gt[:], table, idx_t[:, c0:c0 + IDX_PER_CALL // 16],
                    IDX_PER_CALL, IDX_PER_CALL, D,
                    single_packet=False, queue_num=qn[0] % NQUEUES,
                )
                qn[0] += 1
                dl_v = dl_t[:, k * CPC:(k + 1) * CPC].unsqueeze(1) \
                    .broadcast_to([128, BLK, CPC])
                w_v = w_t[:, k * CPC:(k + 1) * CPC].unsqueeze(1) \
                    .broadcast_to([128, BLK, CPC])
                nc.vector.tensor_tensor(oh[:], iotab_t[:], dl_v, eq)
                nc.vector.tensor_tensor(oh[:], oh[:], w_v, mul_)
                call_tiles[key] = (gt, oh)
                return call_tiles[key]

            chunk_pos = [0, 0]  # next chunk index per half
            nm_t = None
            ps = None
            for bl in range(NBLK):
                g, sub = divmod(bl, GRP // BLK)
                gw = min(GRP, NPAD - g * GRP)
                if sub == 0:
                    ps = psA.tile([128, gw], f32, tag="agg")
                    nm_t = nmp.tile([128, gw], f16, tag="nm")
                col = sub * BLK
                nch_bl = int(capA[bl]) + int(capB[bl])
                done = 0
                for half, cap in ((0, capA[bl]), (1, capB[bl])):
                    for _ in range(int(cap)):
                        j = chunk_pos[half]
                        chunk_pos[half] += 1
                        gt, oh = ensure_call(half, j // CPC)
                        nc.tensor.matmul(
                            ps[:, col:col + BLK],
                            gt[:, j % CPC, :], oh[:, :, j % CPC],
                            start=(done == 0), stop=(done == nch_bl - 1),
                        )
                        done += 1

                if (sub + 1) * BLK == gw:  # group complete
                    nc.vector.tensor_copy(nm_t[:], ps[:])
                    g0 = g * GRP
                    pd = psD.tile([128, gw], f32, tag="dense")
                    nc.tensor.matmul(pd[:], WgT_t[:], xT_t[:, g0:g0 + gw],
                                     start=True, stop=False)
                    nc.tensor.matmul(pd[:], WsT_t[:], xTm_t[:, g0:g0 + gw],
                                     start=False, stop=False)
                    nc.tensor.matmul(pd[:], WlT_t[:], nm_t[:],
                                     start=False, stop=False)
                    nc.tensor.matmul(pd[:], bias_t[:], ones_t[:, :gw],
                                     start=False, stop=True)
                    xm = tmp.tile([128, gw], f32, tag="xm")
                    nc.vector.tensor_scalar(xm[:], pd[:], 0.0, None, min_)
                    ex = tmp.tile([128, gw], f32, tag="ex")
                    nc.scalar.activation(ex[:], xm[:],
                                         mybir.ActivationFunctionType.Exp)
                    r1 = tmp.tile([128, gw], f32, tag="r1")
                    nc.vector.tensor_scalar(r1[:], pd[:], 0.0, -1.0, max_, add_)
                    nc.vector.tensor_tensor(out_t[:, g0:g0 + gw], r1[:], ex[:], add_)

            nc.sync.dma_start(out=out_d[:], in_=out_t[:])

    nc.compile()
    return nc


def kernel(x, Wg, Wl, Ws, b, src, dst, deg, _trace=False):
    from concourse.bass_utils import run_bass_kernel_spmd

    structure, in_maps, block_order = _build_host_data(x, Wg, Wl, Ws, b, src, dst, deg)
    if structure not in _cache:
        _cache[structure] = _build_program(structure)
    nc = _cache[structure]

    kwargs = {}
    if _trace:
        import types, importlib.util
        if importlib.util.find_spec("antenv.axon_hooks") is None:
            mod = types.ModuleType("antenv.axon_hooks")
            mod._hook = None
            mod.set_axon_ntff_profile_hook = lambda h: setattr(mod, "_hook", h)
            mod.get_axon_ntff_profile_hook = lambda: mod._hook
            sys.modules["antenv.axon_hooks"] = mod
            import antenv
            antenv.axon_hooks = mod
            from trn_agent_boot.trn_boot import _ntff_profile_via_ctypes
            mod.set_axon_ntff_profile_hook(
                _ntff_profile_via_ctypes("/opt/axon/libaxon_pjrt.so"))
        from concourse import bass_utils as _bu
        _bu.upload_artifacts = lambda tmpdir: tmpdir
        kwargs["trace"] = True

    try:
        res = run_bass_kernel_spmd(nc, in_maps, list(range(NCORES)), **kwargs)
    except Exception:
        # transient NRT device errors have been observed after heavy
        # back-to-back runs; one retry recovers
        import time
        time.sleep(2.0)
        res = run_bass_kernel_spmd(nc, in_maps, list(range(NCORES)), **kwargs)

    out = np.empty((N, D), np.float32)
    for c in range(NCORES):
        oT = res.results[c]["outT"]
        for pos in range(NBLK):
            bl = int(block_order[c, pos])
            blkw = min(BLK, NPC - bl * BLK)
            out[c * NPC + bl * BLK: c * NPC + bl * BLK + blkw] = \
                oT[:, pos * BLK: pos * BLK + blkw].T
    if _trace:
        kernel.last_exec_time_ns = res.exec_time_ns
    return out
